# revision 31
# baseline (speedup 1.0000x reference)
"""Trainium2 Bass kernel: BiLSTM dependency-parser edge scorer (v2).

Self-contained. Accepts FULL inputs (as produced by setup_inputs()), returns
the FULL [65025, 1] float32 score tensor.

Per-core program (SPMD over 8 cores; cores differ only in the selT input):
  - embeddings gathered on device via indirect DMA, transposed to K-major
  - input projections xg[t] for each (layer, dir) precomputed as GEMMs
    (biases folded in during the PSUM->SBUF copy via per-partition scalars)
  - recurrence state layout: h tile [128, 4] with h[p, j] = h_{128*j+p};
    the j-columns ARE the K-chunks of the next step's matvec, so no
    transpose appears in the recurrent loop. Gate matvec emitted as 64
    single-column matmuls ([128,1] out each) + one [128,16] identity matmul
    injecting xg[t]; gate tile cols = 4*gate + j.
  - cell update: sigmoid over all gates (g-gate pre-scaled by 2 so
    tanh(g) = 2*sigmoid(2g) - 1), fused DVE (2*sg_g - 1)*sg_i, Pool f*c,
    DVE add, Act tanh, DVE o*th written straight into the layer output
    buffer HT[dl] (strided columns), which feeds both the next step's
    matmuls and the next layer's xg GEMM.
  - Edge MLP factored: scores[h,m] = w2 . tanh(A[h] + B[m] + b1) + b2 with
    A = h1 @ Uh^T, B = h1 @ Um^T. Each core computes a [32, 256] slice of
    the score grid (rows picked by the per-core selT one-hot); the host
    assembles and compacts to edge order.
"""

import os
import sys

sys.path.insert(0, "/opt/trn_rl_repo")

import numpy as np

import concourse.bass as bass
import concourse.mybir as mybir
from concourse import bacc
from concourse.bass import IndirectOffsetOnAxis
from concourse.masks import make_identity
from concourse.tile import TileContext

N = 256          # sequence length
HID = 400        # hidden size per direction
NC = 8           # cores
F32 = mybir.dt.float32
BF16 = mybir.dt.float16
I32 = mybir.dt.int32
AF = mybir.ActivationFunctionType
OP = mybir.AluOpType

STEPS = int(os.environ.get("DP_STEPS", str(N)))
USE_AFFINE_MUL = os.environ.get("DP_AFFINE", "1") == "1"
FP8 = os.environ.get("DP_FP8", "0") == "1"
F8 = mybir.dt.float8e4
WSCALE = 32.0


# ---------------------------------------------------------------------------
# host-side weight layout prep
# ---------------------------------------------------------------------------


def _bf(a):
    return np.ascontiguousarray(np.asarray(a).astype(np.float16))


def _f8(a):
    import ml_dtypes
    return np.ascontiguousarray(np.asarray(a).astype(ml_dtypes.float8_e4m3))


def _gate_pad(W, din_pad):
    """W: [1600, Din] torch-gate-order (i,f,g,o). Returns [4, 512, din_pad]
    zero-padded per-gate blocks with the g-gate scaled by 2."""
    Ws = np.array(W, dtype=np.float64)
    Ws[800:1200] *= 2.0
    din = Ws.shape[1]
    out = np.zeros((4, 512, din_pad), np.float64)
    for g in range(4):
        out[g, :HID, :din] = Ws[400 * g: 400 * g + 400]
    return out


def _chunked_lhsT(Wp, nkc):
    """Wp: [4, 512, 128*nkc]. Returns [nkc, 128, 2048] with
    out[kc, k, (4*gate+j)*128 + m] = Wp[gate, 128*j+m, 128*kc+k]."""
    nb = Wp.shape[2] // 128
    assert nb == nkc
    # [4, 4, 128, nkc, 128] = (gate, j, m, kc, k)
    W5 = Wp.reshape(4, 4, 128, nkc, 128)
    # -> (kc, k, gate, j, m)
    return np.ascontiguousarray(W5.transpose(3, 4, 0, 1, 2).reshape(nkc, 128, 2048))


def _fold_in_pad(U):
    """U: [800, M] (h1cat-dim major). Returns [1024, M] with each 400-dim
    half zero-padded to 512."""
    M = U.shape[1]
    out = np.zeros((1024, M), np.float64)
    out[0:400] = U[0:400]
    out[512:912] = U[400:800]
    return out


def _prep_inputs(word_idx, pos_idx, word_emb, pos_emb,
                 Wih0, Whh0, bih0, bhh0, Wih1, Whh1, bih1, bhh1,
                 fc1_W, fc1_b, fc2_W, fc2_b):
    arr = {}
    arr["widx"] = np.ascontiguousarray(
        np.asarray(word_idx).reshape(N, 1).astype(np.int32))
    arr["pidx"] = np.ascontiguousarray(
        np.asarray(pos_idx).reshape(N, 1).astype(np.int32))
    arr["wemb"] = np.ascontiguousarray(np.asarray(word_emb, dtype=np.float32))
    arr["pemb"] = np.ascontiguousarray(np.asarray(pos_emb, dtype=np.float32))

    Wih = [np.asarray(Wih0, np.float64), np.asarray(Wih1, np.float64)]
    Whh = [np.asarray(Whh0, np.float64), np.asarray(Whh1, np.float64)]
    bih = [np.asarray(bih0, np.float64), np.asarray(bih1, np.float64)]
    bhh = [np.asarray(bhh0, np.float64), np.asarray(bhh1, np.float64)]

    whh = np.zeros((4, 4, 128, 2048), np.float64)
    xgb = np.zeros((4, 128, 16), np.float64)
    for l in range(2):
        for d in range(2):
            dl = 2 * l + d
            whh[dl] = _chunked_lhsT(_gate_pad(Whh[l][d], 512), 4)
            bs = np.array(bih[l][d] + bhh[l][d], np.float64)
            bs[800:1200] *= 2.0
            for g in range(4):
                for j in range(4):
                    nvalid = min(128, HID - 128 * j)
                    xgb[dl, 0:nvalid, 4 * g + j] = \
                        bs[400 * g + 128 * j: 400 * g + 128 * j + nvalid]
    if FP8:
        # weights x32 in fp8 (sigmoid compensates with scale=1/32);
        # xg path (wih/xgb) also x32 so the PSUM gates add consistently
        arr["whh"] = _f8(whh * WSCALE)
        arr["xgb"] = np.ascontiguousarray((xgb * WSCALE).astype(np.float32))
    else:
        arr["whh"] = _bf(whh)
        arr["xgb"] = np.ascontiguousarray(xgb.astype(np.float32))

    wih0 = np.zeros((2, 4, 128, 2048), np.float64)
    wih1 = np.zeros((2, 8, 128, 2048), np.float64)
    for d in range(2):
        wih0[d] = _chunked_lhsT(_gate_pad(Wih[0][d], 512), 4)
        # layer-1 input is h0cat (800) in padded layout (1024)
        Wp = _gate_pad(Wih[1][d], 1024)
        W1p = np.zeros_like(Wp)
        W1p[:, :, 0:400] = Wp[:, :, 0:400]
        W1p[:, :, 512:912] = Wp[:, :, 400:800]
        wih1[d] = _chunked_lhsT(W1p, 8)
    if FP8:
        wih0 *= WSCALE
        wih1 *= WSCALE
    arr["wih0"] = _bf(wih0)
    arr["wih1"] = _bf(wih1)

    # edge MLP weights
    f1 = np.asarray(fc1_W, np.float64)                  # [100, 1600]
    UhP = _fold_in_pad(f1[:, :800].T)                   # [1024, 100]
    UmP = _fold_in_pad(f1[:, 800:].T)
    arr["uhT"] = _bf(UhP.reshape(8, 128, 100).transpose(1, 0, 2).reshape(128, 800))
    arr["umT"] = _bf(UmP.reshape(8, 128, 100).transpose(1, 0, 2).reshape(128, 800))
    arr["w2e"] = _bf(np.asarray(fc2_W, np.float32).reshape(100, 1))
    arr["b1"] = np.ascontiguousarray(
        np.asarray(fc1_b, np.float32).reshape(100, 1))
    arr["b2"] = np.ascontiguousarray(
        np.full((128, 1), np.float32(np.asarray(fc2_b).reshape(())),
                dtype=np.float32))
    arr["eye"] = _bf(np.eye(128, dtype=np.float32))
    return arr


def _make_selT(core):
    s = np.zeros((2, 128, 32), np.float32)
    for r in range(32):
        t = 32 * core + r
        s[t // 128, t % 128, r] = 1.0
    return _bf(s)


# ---------------------------------------------------------------------------
# device kernel build
# ---------------------------------------------------------------------------


def _emit_xg(nc, l, wih_sb, XG, XT_chunks, xgb_sb, xg_ps):
    """xg GEMM for layer l. XT_chunks(d, kc) -> (rhs_ap, kmax).
    Output XG[d] layout: [128, 16*256] bf16, block oc=(4*gate+j) at cols
    oc*256 + t, bias folded via per-partition scalar add in the copy."""
    nkc = 4 if l == 0 else 8
    for d in range(2):
        dl = 2 * l + d
        for oc in range(16):
            ps = xg_ps.tile([128, 512], F32, name="xgps", tag="xgps")
            for kc in range(nkc):
                rhs, kmax = XT_chunks(d, kc)
                nc.tensor.matmul(
                    ps[0:128, 0:256],
                    lhsT=wih_sb[d][kc][0:kmax, 128 * oc: 128 * oc + 128],
                    rhs=rhs,
                    start=(kc == 0), stop=(kc == nkc - 1))
            nc.vector.tensor_scalar(
                out=XG[d][0:128, 256 * oc: 256 * oc + 256],
                in0=ps[0:128, 0:256],
                scalar1=xgb_sb[0:128, 16 * dl + oc: 16 * dl + oc + 1],
                scalar2=None, op0=OP.add)


def _emit_rec(nc, l, whh_sb, XG, HT, pools):
    """Emit STEPS wall-steps for layer l (both directions interleaved).

    Gates accumulate in a PSUM bank tile (cols 0:16, = 4*gate + j).
    c and tanh(c) live in a second PSUM bank tile `cth` (cols 0:4 = c,
    4:8 = th): Pool reads/writes PSUM with no access-latency penalty and
    the Act tanh runs PSUM->PSUM (cheaper init than SBUF).
    """
    psp, cthp, sgp, t1p, cfp, accp = pools
    prev_cth = [None, None]
    for step in range(STEPS):
        for d in range(2):
            dl = 2 * l + d
            t = step if d == 0 else STEPS - 1 - step
            ps = psp[d].tile([128, 512], F32, name=f"ps{d}", tag=f"ps{d}")
            cth = cthp[d].tile([128, 8], F32, name=f"ct{d}", tag=f"ct{d}")
            # xg[t] injection: gates = I @ xg_t (also opens the accum group)
            nc.tensor.matmul(
                ps[0:128, 0:16],
                lhsT=nc_eye(nc)[0:128, 0:128],
                rhs=XG[d][0:128, t: t + 15 * 256 + 1: 256],
                start=True, stop=(step == 0),
                skip_group_check=True)
            tprev = t - 1 if d == 0 else t + 1

            def w_mms(oc0, oc1):
                if FP8:
                    # DoubleRow: two K-chunks per matmul (fp8 only)
                    for q in range(2):
                        rhs = HT[dl][0:128, 2 * q: 2 * q + 2,
                                     tprev: tprev + 1]
                        for oc in range(oc0, oc1):
                            nc.tensor.matmul(
                                ps[0:128, oc: oc + 1],
                                lhsT=whh_sb[dl][0:128, 2 * q: 2 * q + 2,
                                                128 * oc: 128 * oc + 128],
                                rhs=rhs,
                                start=False, stop=(q == 1),
                                perf_mode=mybir.MatmulPerfMode.DoubleRow,
                                skip_group_check=True)
                else:
                    for kc in range(4):
                        rhs = HT[dl][0:128, kc, tprev: tprev + 1]
                        for oc in range(oc0, oc1):
                            nc.tensor.matmul(
                                ps[0:128, oc: oc + 1],
                                lhsT=whh_sb[dl][0:128, kc,
                                                128 * oc: 128 * oc + 128],
                                rhs=rhs,
                                start=False, stop=(kc == 3),
                                skip_group_check=True)

            # sigmoid over the gates (g rows pre-scaled by 2); o-gate
            # matmuls + sigmoid trail the i/f/g ones so the critical path
            # waits on 48 matmuls, not 64
            sg = sgp.tile([128, 16], F32, name=f"sg{d}", tag=f"sg{d}")
            if step > 0:
                w_mms(0, 12)
            nc.scalar.activation(sg[0:128, 0:12], ps[0:128, 0:12], AF.Sigmoid,
                                 scale=(1.0 / WSCALE) if FP8 else 1.0)
            if step > 0:
                w_mms(12, 16)
            nc.scalar.activation(sg[0:128, 12:16], ps[0:128, 12:16],
                                 AF.Sigmoid,
                                 scale=(1.0 / WSCALE) if FP8 else 1.0)
            # cell update: cf on Pool (fresh Act wait rides first position),
            # t1 on DVE (proven custom op), add on Pool (its only cross wait
            # is the DVE t1 sem, so it parks pre-decoded in the queue)
            if step > 0:
                cf = cfp.tile([128, 4], F32, name=f"cf{d}", tag=f"cf{d}")
                t1 = t1p.tile([128, 4], F32, name=f"t1{d}", tag=f"t1{d}")
                acc = accp.tile([128, 1], F32, name=f"ac{d}", tag=f"ac{d}")
                nc.gpsimd.tensor_tensor(
                    out=cf[0:128, 0:4], in0=sg[0:128, 4:8],
                    in1=prev_cth[d][0:128, 0:4], op=OP.mult)
                nc.vector.affine_mul_reduce(
                    out=t1[0:128, 0:4], accum_out=acc[0:128, 0:1],
                    in0=sg[0:128, 8:12], in1=sg[0:128, 0:4],
                    scale=2.0, bias=-1.0)
                nc.gpsimd.tensor_tensor(
                    out=cth[0:128, 0:4], in0=t1[0:128, 0:4],
                    in1=cf[0:128, 0:4], op=OP.add)
            else:
                acc = accp.tile([128, 1], F32, name=f"ac{d}", tag=f"ac{d}")
                nc.vector.affine_mul_reduce(
                    out=cth[0:128, 0:4], accum_out=acc[0:128, 0:1],
                    in0=sg[0:128, 8:12], in1=sg[0:128, 0:4],
                    scale=2.0, bias=-1.0)
            # th = tanh(c) (Act, PSUM->PSUM)
            nc.scalar.activation(cth[0:128, 4:8], cth[0:128, 0:4], AF.Tanh)
            # h = th * sg_o -> HT (Pool)
            nc.gpsimd.tensor_tensor(
                out=HT[dl][0:128, 0:4, t: t + 1],
                in0=cth[0:128, 4:8], in1=sg[0:128, 12:16], op=OP.mult)
            prev_cth[d] = cth


_EYE_SB = None


def nc_eye(nc):
    return _EYE_SB


def build_nc():
    global _EYE_SB
    nc = bacc.Bacc("TRN2", target_bir_lowering=False, debug=False,
                   num_devices=NC)
    # ---- DRAM parameters ----
    wemb = nc.dram_tensor("wemb", [50000, 300], F32, kind="ExternalInput").ap()
    pemb = nc.dram_tensor("pemb", [50, 100], F32, kind="ExternalInput").ap()
    widx = nc.dram_tensor("widx", [N, 1], I32, kind="ExternalInput").ap()
    pidx = nc.dram_tensor("pidx", [N, 1], I32, kind="ExternalInput").ap()
    whhD = nc.dram_tensor("whh", [4, 4, 128, 2048], F8 if FP8 else BF16,
                      kind="ExternalInput").ap()
    wih0D = nc.dram_tensor("wih0", [2, 4, 128, 2048], BF16, kind="ExternalInput").ap()
    wih1D = nc.dram_tensor("wih1", [2, 8, 128, 2048], BF16, kind="ExternalInput").ap()
    xgbD = nc.dram_tensor("xgb", [4, 128, 16], F32, kind="ExternalInput").ap()
    uhTD = nc.dram_tensor("uhT", [128, 800], BF16, kind="ExternalInput").ap()
    umTD = nc.dram_tensor("umT", [128, 800], BF16, kind="ExternalInput").ap()
    w2eD = nc.dram_tensor("w2e", [100, 1], BF16, kind="ExternalInput").ap()
    b1D = nc.dram_tensor("b1", [100, 1], F32, kind="ExternalInput").ap()
    b2D = nc.dram_tensor("b2", [128, 1], F32, kind="ExternalInput").ap()
    eyeD = nc.dram_tensor("eye", [128, 128], BF16, kind="ExternalInput").ap()
    selTD = nc.dram_tensor("selT", [2, 128, 32], BF16, kind="ExternalInput").ap()
    grid = nc.dram_tensor("grid", [32, N], F32, kind="ExternalOutput").ap()

    from contextlib import ExitStack
    with TileContext(nc) as tc, ExitStack() as ctx:
        top = ctx.enter_context(tc.tile_pool(name="top", bufs=1))
        # ---- persistent tiles ----
        eye_sb = top.tile([128, 128], BF16, name="eye", tag="eye")
        nc.sync.dma_start(out=eye_sb[:, :], in_=eyeD[:, :])
        _EYE_SB = eye_sb
        xgb_sb = top.tile([128, 64], F32, name="xgb", tag="xgb")
        for dl in range(4):
            nc.sync.dma_start(out=xgb_sb[0:128, 16 * dl: 16 * dl + 16],
                              in_=xgbD[dl])
        idn = top.tile([128, 128], F32, name="idn", tag="idn")
        make_identity(nc, idn[:, :])
        HT = [top.tile([128, 4, 256], F8 if FP8 else BF16,
                       name=f"HT{dl}", tag=f"HT{dl}")
              for dl in range(4)]
        if STEPS < N:
            for dl in range(4):
                nc.gpsimd.memset(HT[dl][:, :, :], 0.0)
        whh_sb = [top.tile([128, 4, 2048], F8 if FP8 else BF16,
                           name=f"whh{dl}", tag=f"whh{dl}")
                  for dl in range(4)]

        # =========== embedding gather + transpose ===========
        XT = top.tile([128, 1024], BF16, name="XT", tag="XT")
        nc.gpsimd.memset(XT[:, :], 0.0)
        with tc.tile_pool(name="embed", bufs=1) as epool, \
             tc.tile_pool(name="embps", bufs=2, space="PSUM") as eps:
            idx_sb = epool.tile([128, 4], I32, name="idx", tag="idx")
            nc.sync.dma_start(out=idx_sb[0:128, 0:1], in_=widx[0:128, 0:1])
            nc.sync.dma_start(out=idx_sb[0:128, 1:2], in_=widx[128:256, 0:1])
            nc.sync.dma_start(out=idx_sb[0:128, 2:3], in_=pidx[0:128, 0:1])
            nc.sync.dma_start(out=idx_sb[0:128, 3:4], in_=pidx[128:256, 0:1])
            x_sb = epool.tile([128, 800], F32, name="xsb", tag="xsb")
            for tc_ in range(2):
                nc.gpsimd.indirect_dma_start(
                    out=x_sb[0:128, 400 * tc_: 400 * tc_ + 300],
                    out_offset=None,
                    in_=wemb[:, :],
                    in_offset=IndirectOffsetOnAxis(
                        ap=idx_sb[0:128, tc_:tc_ + 1], axis=0))
                nc.gpsimd.indirect_dma_start(
                    out=x_sb[0:128, 400 * tc_ + 300: 400 * tc_ + 400],
                    out_offset=None,
                    in_=pemb[:, :],
                    in_offset=IndirectOffsetOnAxis(
                        ap=idx_sb[0:128, 2 + tc_:3 + tc_], axis=0))
            for tc_ in range(2):
                for kc in range(4):
                    w = 128 if kc < 3 else 16
                    ptr = eps.tile([128, 128], F32, name="ptr", tag="ptr")
                    nc.tensor.transpose(
                        out=ptr[0:w, 0:128],
                        in_=x_sb[0:128, 400 * tc_ + 128 * kc:
                                 400 * tc_ + 128 * kc + w],
                        identity=idn[:, :])
                    nc.vector.tensor_copy(
                        out=XT[0:w, 256 * kc + 128 * tc_:
                               256 * kc + 128 * tc_ + 128],
                        in_=ptr[0:w, 0:128])

        # weight DMAs, in order of first use (single contended DMA device):
        # wih0 (xg0), whh (rec0), wih1 (xg1 - overlaps rec0)
        l0w = ctx.enter_context(tc.tile_pool(name="l0w", bufs=1))
        wih0_sb = [[l0w.tile([128, 2048], BF16, name=f"w0{d}{kc}",
                             tag=f"w0{d}{kc}") for kc in range(4)]
                   for d in range(2)]
        for d in range(2):
            for kc in range(4):
                nc.sync.dma_start(out=wih0_sb[d][kc][:, :], in_=wih0D[d, kc])
        for dl in range(4):
            for kc in range(4):
                nc.sync.dma_start(
                    out=whh_sb[dl][0:128, kc: kc + 1, 0:2048],
                    in_=whhD[dl, kc])
        l1w = ctx.enter_context(tc.tile_pool(name="l1w", bufs=1))
        wih1_sb = [[l1w.tile([128, 2048], BF16, name=f"w1{d}{kc}",
                             tag=f"w1{d}{kc}") for kc in range(8)]
                   for d in range(2)]
        for d in range(2):
            for kc in range(8):
                nc.sync.dma_start(out=wih1_sb[d][kc][:, :], in_=wih1D[d, kc])

        # shared small pools for both recurrences
        rec = ctx.enter_context(ExitStack())
        sgp = rec.enter_context(tc.tile_pool(name="sg", bufs=3))
        t1p = rec.enter_context(tc.tile_pool(name="t1", bufs=3))
        cfp = rec.enter_context(tc.tile_pool(name="cf", bufs=3))
        accp = rec.enter_context(tc.tile_pool(name="acc", bufs=3))

        # =========== layer 0: xg + recurrence ===========
        with tc.tile_pool(name="xg0", bufs=1) as xg0p:
            XG0 = [xg0p.tile([128, 4096], BF16, name=f"XG0{d}", tag=f"XG0{d}")
                   for d in range(2)]

            def xt_chunks(d, kc):
                kmax = 128 if kc < 3 else 16
                return XT[0:kmax, 256 * kc: 256 * kc + 256], kmax

            with tc.tile_pool(name="xgps0", bufs=4, space="PSUM") as xg_ps:
                _emit_xg(nc, 0, wih0_sb, XG0, xt_chunks, xgb_sb, xg_ps)
            with tc.tile_pool(name="ps0", bufs=2, space="PSUM") as ps0, \
                 tc.tile_pool(name="ps1", bufs=2, space="PSUM") as ps1, \
                 tc.tile_pool(name="ct0", bufs=3) as ct0, \
                 tc.tile_pool(name="ct1", bufs=3) as ct1:
                _emit_rec(nc, 0, whh_sb, XG0, HT,
                          ([ps0, ps1], [ct0, ct1], sgp, t1p, cfp, accp))

        # =========== layer 1: xg + recurrence ===========
        with tc.tile_pool(name="xg1", bufs=1) as xg1p:
            XG1 = [xg1p.tile([128, 4096], BF16, name=f"XG1{d}", tag=f"XG1{d}")
                   for d in range(2)]

            def h0_chunks(d, kc):
                kmax = 128 if (kc % 4) < 3 else 16
                return (HT[kc // 4][0:kmax, kc % 4, 0:256], kmax)

            with tc.tile_pool(name="xgps1", bufs=4, space="PSUM") as xg_ps:
                _emit_xg(nc, 1, wih1_sb, XG1, h0_chunks, xgb_sb, xg_ps)
            with tc.tile_pool(name="ps0", bufs=2, space="PSUM") as ps0, \
                 tc.tile_pool(name="ps1", bufs=2, space="PSUM") as ps1, \
                 tc.tile_pool(name="ct0", bufs=3) as ct0, \
                 tc.tile_pool(name="ct1", bufs=3) as ct1:
                _emit_rec(nc, 1, whh_sb, XG1, [None, None, HT[2], HT[3]],
                          ([ps0, ps1], [ct0, ct1], sgp, t1p, cfp, accp))
        rec.close()

        # =========== edge scorer ===========
        with tc.tile_pool(name="edge", bufs=1) as ep, \
             tc.tile_pool(name="edgeth", bufs=3) as thp2, \
             tc.tile_pool(name="edgeps", bufs=1, space="PSUM") as epps, \
             tc.tile_pool(name="edgepsS", bufs=1, space="PSUM") as spps:
            uhT_sb = ep.tile([128, 800], BF16, name="uhT", tag="uhT")
            nc.sync.dma_start(out=uhT_sb[:, :], in_=uhTD[:, :])
            umT_sb = ep.tile([128, 800], BF16, name="umT", tag="umT")
            nc.sync.dma_start(out=umT_sb[:, :], in_=umTD[:, :])
            w2e_sb = ep.tile([100, 1], BF16, name="w2e", tag="w2e")
            nc.sync.dma_start(out=w2e_sb[:, :], in_=w2eD[:, :])
            b1_sb = ep.tile([100, 1], F32, name="b1", tag="b1")
            nc.sync.dma_start(out=b1_sb[:, :], in_=b1D[:, :])
            b2_sb = ep.tile([128, 1], F32, name="b2", tag="b2")
            nc.sync.dma_start(out=b2_sb[:, :], in_=b2D[:, :])
            selT_sb = ep.tile([128, 64], BF16, name="selT", tag="selT")
            nc.sync.dma_start(out=selT_sb[0:128, 0:32], in_=selTD[0])
            nc.sync.dma_start(out=selT_sb[0:128, 32:64], in_=selTD[1])

            # A2 [t(128 x 2 chunks), 100]
            A2_sb = ep.tile([128, 200], BF16, name="A2", tag="A2")
            for m in range(2):
                pA = epps.tile([128, 512], F32, name="pA", tag="pA")
                for kc in range(8):
                    kmax = 128 if (kc % 4) < 3 else 16
                    nc.tensor.matmul(
                        pA[0:128, 0:100],
                        lhsT=HT[2 + kc // 4][0:kmax, kc % 4,
                                             128 * m: 128 * m + 128],
                        rhs=uhT_sb[0:kmax, 100 * kc: 100 * kc + 100],
                        start=(kc == 0), stop=(kc == 7))
                nc.vector.tensor_copy(out=A2_sb[0:128, 100 * m: 100 * m + 100],
                                      in_=pA[0:128, 0:100])
            # Asel = selT^T @ A2 -> [32, 100] -> transpose -> ATb [100, 32]
            AselS = ep.tile([128, 128], F32, name="AselS", tag="AselS")
            nc.gpsimd.memset(AselS[:, :], 0.0)
            pS = epps.tile([128, 512], F32, name="pS", tag="pS")
            for m in range(2):
                nc.tensor.matmul(
                    pS[0:32, 0:100],
                    lhsT=selT_sb[0:128, 32 * m: 32 * m + 32],
                    rhs=A2_sb[0:128, 100 * m: 100 * m + 100],
                    start=(m == 0), stop=(m == 1))
            nc.vector.tensor_copy(out=AselS[0:32, 0:100], in_=pS[0:32, 0:100])
            pAT = epps.tile([128, 512], F32, name="pAT", tag="pAT")
            nc.tensor.transpose(out=pAT[0:128, 0:128],
                                in_=AselS[0:128, 0:128], identity=idn[:, :])
            ATb = ep.tile([128, 32], F32, name="ATb", tag="ATb")
            nc.vector.tensor_scalar(
                out=ATb[0:100, 0:32], in0=pAT[0:100, 0:32],
                scalar1=b1_sb[0:100, 0:1], scalar2=None, op0=OP.add)
            # BT [100, 256]
            BT_sb = ep.tile([128, 256], BF16, name="BT", tag="BT")
            pB = epps.tile([128, 512], F32, name="pB", tag="pB")
            for kc in range(8):
                kmax = 128 if (kc % 4) < 3 else 16
                nc.tensor.matmul(
                    pB[0:100, 0:256],
                    lhsT=umT_sb[0:kmax, 100 * kc: 100 * kc + 100],
                    rhs=HT[2 + kc // 4][0:kmax, kc % 4, 0:256],
                    start=(kc == 0), stop=(kc == 7))
            nc.vector.tensor_copy(out=BT_sb[0:100, 0:256], in_=pB[0:100, 0:256])

            # per-row tanh + w2 dot
            psS_tiles = [spps.tile([128, 512], F32, name=f"psS{q}", tag=f"psS{q}")
                         for q in range(4)]
            for q in range(4):
                nc.vector.memset(psS_tiles[q][:, :], 0.0)
            gsb_tiles = [ep.tile([128, 512], F32, name=f"gsb{q}", tag=f"gsb{q}")
                         for q in range(4)]
            for r in range(32):
                th_t = thp2.tile([128, 256], BF16, name="tht", tag="tht")
                nc.scalar.activation(
                    th_t[0:100, 0:256], BT_sb[0:100, 0:256], AF.Tanh,
                    bias=ATb[0:100, r:r + 1], scale=1.0)
                q, half = divmod(r // 4, 2)
                nc.tensor.matmul(
                    psS_tiles[q][32 * (r % 4): 32 * (r % 4) + 1,
                                 256 * half: 256 * half + 256],
                    lhsT=w2e_sb[0:100, 0:1],
                    rhs=th_t[0:100, 0:256],
                    start=True, stop=True,
                    skip_group_check=True,
                    tile_position=(0, 32 * (r % 4)))
            for q in range(4):
                nc.vector.tensor_scalar(
                    out=gsb_tiles[q][0:128, 0:512],
                    in0=psS_tiles[q][0:128, 0:512],
                    scalar1=b2_sb[0:128, 0:1], scalar2=None, op0=OP.add)
                for half in range(2):
                    rb = 4 * (2 * q + half)
                    nc.sync.dma_start(
                        out=grid[rb:rb + 4, 0:256],
                        in_=gsb_tiles[q][0:128:32, 256 * half: 256 * half + 256])

    nc.compile()
    return nc


_NC_CACHE = None


def _get_nc():
    global _NC_CACHE
    if _NC_CACHE is None:
        _NC_CACHE = build_nc()
    return _NC_CACHE


def kernel(**inputs) -> np.ndarray:
    from concourse.bass_utils import run_bass_kernel_spmd

    arr = _prep_inputs(**inputs)
    nc = _get_nc()
    in_maps = []
    for k in range(NC):
        m = dict(arr)
        m["selT"] = _make_selT(k)
        in_maps.append(m)
    res = run_bass_kernel_spmd(nc, in_maps, core_ids=list(range(NC)))
    grid = np.concatenate([res.results[k]["grid"] for k in range(NC)], axis=0)
    mask = np.ones((N, N), dtype=bool)
    np.fill_diagonal(mask, False)
    mask[:, 0] = False
    return grid[mask].reshape(-1, 1).astype(np.float32)
